# revision 33
# baseline (speedup 1.0000x reference)
"""Trainium2 Bass kernel for the AnnRC spiking-RNN problem.

Strategy: pure data parallelism across batch (8 cores x 32 rows each).

Input matmul (C = 0.5*(x @ W_in + bias), all frames): THREE accumulating
fp16 passes reconstruct the exact-fp32 product at 1 cyc/row (4x the
fp32 rate):  fp16(x)@fp16(W) + fp16(x-fp16(x))@fp16(W) +
fp16(x)@fp16(W-fp16(W)).  The lo-parts are fp16 subnormals, which the
PE multiplies exactly (verified on hw); total U error ~7e-7 rel -> ~40
spike flips vs the ~476-flip budget (rel-err gate 2e-2).  Streamed to
DRAM in a "quad" layout [(hc,b), n] and interleaved with the frame loop
(one (b,t)-chunk per 4 frames) to fill PE slots and keep HAM warm.

Recurrence per frame:
  r' = spike @ (0.5*A)  -- A as an exact bf16 hi+lo split, two
       accumulating passes (single-pass fp16/bf16 A fails: the spike
       cascade is supercritical; fp16-A alone gives ~1964 flips,
       rel 4e-2 -- measured on hw AND reproduced exactly by the numpy
       sim in sim_precision.py). The four 512-wide output chunks run
       CONCURRENTLY in the four column groups of the PE array via
       tile_position (M=32 each), quadrupling effective throughput of
       the M=32 stationary.  NOTE: f32r (11-bit RNE operands, 1 cyc/row
       at N>=256) cannot be used here -- the ISA forbids tile_position
       col-tiling for 4-byte dtypes (s3d3_mm_valid_dst_partition).
  Per 128-wide block j (pipelined so the next frame's matmuls can start
  on early blocks):  t = r' + C_t (DVE add from PSUM);  full 128x128 PE
  transpose into packed hidden-major order (h = hc*512 + j*128 + p);
  y = tanh (ACT, reading PSUM);  spike' = (y > 0.5 - h)  (DVE is_gt,
  bf16 out -> feeds the next frame's stationary directly).
  Off the critical chain: mem' = y + h, h' = 0.5*mem' + 0.5*spike' - 0.5.

State is hidden-major packed [128, (j, hc, b)] so elementwise ops use all
128 partitions and spike slices feed the matmul stationary without any
per-frame transposition. Outputs are written packed and unpacked on host.
"""

import os
import sys
import types

import numpy as np

# ---------------------------------------------------------------------------
# antenv.axon_hooks shim: this image's antenv lacks the module, and
# concourse.bass_utils imports it unconditionally when tracing is requested.
if "antenv.axon_hooks" not in sys.modules:
    _hooks_mod = types.ModuleType("antenv.axon_hooks")
    _hooks_mod._hook = None
    _hooks_mod.set_axon_ntff_profile_hook = lambda h: setattr(_hooks_mod, "_hook", h)
    _hooks_mod.get_axon_ntff_profile_hook = lambda: _hooks_mod._hook
    sys.modules["antenv.axon_hooks"] = _hooks_mod
    try:
        from trn_agent_boot.trn_boot import _ntff_profile_via_ctypes

        _hooks_mod._hook = _ntff_profile_via_ctypes("/opt/axon/libaxon_pjrt.so")
    except Exception:
        pass

import concourse.bacc as bacc
import concourse.bass_utils as bass_utils
import concourse.mybir as mybir
import concourse.tile as tile
from concourse.bass_utils import run_bass_kernel_spmd

# Zero-egress container: artifact upload would fail; keep local.
bass_utils.upload_artifacts = lambda tmpdir: tmpdir

ALPHA, DECAY, THR = 0.5, 0.5, 0.5
N_IN, N_HID = 700, 2048
BATCH, FRAMES = 256, 128
NCORES = 8
B = BATCH // NCORES          # 32 batch rows per core
KT = N_HID // 128            # 16 k-tiles of the recurrent contraction
HC = N_HID // 512            # 4 n-chunks of 512
KIN = 768                    # padded input contraction (700 + 1 bias + pad)
KC = KIN // 128              # 6 k-tiles for the input matmul
BT = B * FRAMES              # 4096 (batch,frame) pairs per core
BTC = BT // 128              # 32 chunks of 128 bt-pairs

F32 = mybir.dt.float32
F32R = mybir.dt.float32r
BF16 = mybir.dt.bfloat16
FP16 = mybir.dt.float16

LAST_RESULT = None  # test.py reads .exec_time_ns off this after a traced call

_NC_CACHE = {}


def _build_nc(frames):
    nc = bacc.Bacc("TRN2", target_bir_lowering=False, debug=False, num_devices=NCORES)

    Aph = nc.declare_dram_parameter("Aph", [128, KT * N_HID], BF16, isOutput=False)
    Apl = nc.declare_dram_parameter("Apl", [128, KT * N_HID], BF16, isOutput=False)
    Wph = nc.declare_dram_parameter("Wph", [128, KC * N_HID], FP16, isOutput=False)
    Wpl = nc.declare_dram_parameter("Wpl", [128, KC * N_HID], FP16, isOutput=False)
    xTh = nc.declare_dram_parameter("xTh", [KIN, BT], FP16, isOutput=False)
    xTl = nc.declare_dram_parameter("xTl", [KIN, BT], FP16, isOutput=False)
    mem0 = nc.declare_dram_parameter("mem0", [128, 512], F32, isOutput=False)
    eye = nc.declare_dram_parameter("eye", [128, 128], F32, isOutput=False)
    memsT = nc.declare_dram_parameter("memsT", [frames, 128, 512], F32, isOutput=True)
    spikesT = nc.declare_dram_parameter("spikesT", [frames, 128, 512], BF16, isOutput=True)

    btc_used = (B * frames + 127) // 128

    with tile.TileContext(nc) as tc:
        with (
            tc.tile_pool(name="dram", bufs=1, space="DRAM") as dram,
            tc.tile_pool(name="state", bufs=1) as st,
            tc.tile_pool(name="big", bufs=1) as big,
        ):
            C_d = dram.tile([FRAMES * 128, 512], F32, tag="C")
            eye_sb = st.tile([128, 128], F32, tag="eye")
            nc.sync.dma_start(eye_sb[:], eye[:])
            mem0_sb = st.tile([128, 512], F32, tag="mem0")
            nc.sync.dma_start(mem0_sb[:], mem0[:])

            # W first (per k-tile so the first matmuls start after ~2MB),
            # then A (only needed from frame 1's recurrence).
            Wh_sb = big.tile([128, KC * N_HID], FP16, tag="Wh")
            Wl_sb = big.tile([128, KC * N_HID], FP16, tag="Wl")
            for kc in range(KC):
                sl = slice(kc * N_HID, (kc + 1) * N_HID)
                nc.sync.dma_start(Wh_sb[:, sl], Wph[:, sl])
                nc.sync.dma_start(Wl_sb[:, sl], Wpl[:, sl])
            A_hi = big.tile([128, KT * N_HID], BF16, tag="Ahi")
            A_lo = big.tile([128, KT * N_HID], BF16, tag="Alo")
            # all of A_hi before any A_lo, in the recurrence's kt consumption
            # order (kt = 4*kk + q, q outer), so frame 1's hi pass starts
            # after ~8.4MB instead of ~33.6MB.
            kt_order = [4 * kk + q for q in range(4) for kk in range(4)]
            for kt in kt_order:
                sl = slice(kt * N_HID, (kt + 1) * N_HID)
                nc.sync.dma_start(A_hi[:, sl], Aph[:, sl])
            for kt in kt_order:
                sl = slice(kt * N_HID, (kt + 1) * N_HID)
                nc.sync.dma_start(A_lo[:, sl], Apl[:, sl])

            with (
                tc.tile_pool(name="ustg", bufs=2) as ustg,
                tc.tile_pool(name="ups", bufs=2, space="PSUM") as ups,
                tc.tile_pool(name="fstg", bufs=2) as fs,
                tc.tile_pool(name="cpool", bufs=2) as cp,
                tc.tile_pool(name="rps", bufs=2, space="PSUM") as rps,
                tc.tile_pool(name="tps", bufs=4, space="PSUM") as tps,
            ):
                def emit_uchunk(btc):
                    # U for (b,t) chunk btc: C rows for frames 4*btc..4*btc+3.
                    # 3 accumulating passes reconstruct exact-f32 x@W from
                    # 11-bit f32r rounding: x@Wh + (x-RN11(x))@Wh + bf16 x@Wl.
                    src_h = xTh[:, btc * 128:(btc + 1) * 128]
                    src_l = xTl[:, btc * 128:(btc + 1) * 128]
                    xch = ustg.tile([128, KIN], FP16, tag="xch", name="xch")
                    nc.sync.dma_start(xch[:], src_h.rearrange("(kc p) j -> p kc j", p=128))
                    xcl = ustg.tile([128, KIN], FP16, tag="xcl", name="xcl")
                    nc.sync.dma_start(xcl[:], src_l.rearrange("(kc p) j -> p kc j", p=128))
                    for hc in range(HC):
                        ut = ups.tile([128, 512], F32, tag="u", name="ut")
                        for p, (xc_, W_) in enumerate(
                            ((xch, Wh_sb), (xcl, Wh_sb), (xch, Wl_sb))
                        ):
                            for kc in range(KC):
                                nc.tensor.matmul(
                                    ut[:],
                                    xc_[:, kc * 128:(kc + 1) * 128],
                                    W_[:, kc * N_HID + hc * 512:
                                       kc * N_HID + hc * 512 + 512],
                                    start=(p == 0 and kc == 0),
                                    stop=(p == 2 and kc == KC - 1),
                                )
                        cst = ustg.tile([128, 512], F32, tag="cst", name="cst", bufs=1)
                        nc.vector.tensor_copy(cst[:], ut[:])
                        for ts in range(4):
                            row = (4 * btc + ts) * 128 + hc * 32
                            nc.sync.dma_start(
                                C_d[row:row + 32, :],
                                cst[ts * 32:(ts + 1) * 32, :],
                            )

                mem_t = [st.tile([128, 512], F32, tag=f"mem{i}", name=f"mem{i}") for i in range(2)]
                spk_t = [st.tile([128, 512], BF16, tag=f"spk{i}", name=f"spk{i}") for i in range(2)]
                h_sb = st.tile([128, 512], F32, tag="h")
                h1_sb = st.tile([128, 512], F32, tag="h1")
                sph_sb = st.tile([128, 512], F32, tag="sph")
                tmh_sb = st.tile([128, 512], F32, tag="tmh")

                # h for frame 0: spike=0 -> h = 0.5*mem0 - 0.5
                nc.vector.tensor_scalar(
                    h_sb[:], mem0_sb[:], 0.5, -0.5,
                    mybir.AluOpType.mult, mybir.AluOpType.add,
                )
                nc.vector.tensor_scalar(
                    tmh_sb[:], h_sb[:], -1.0, 0.5,
                    mybir.AluOpType.mult, mybir.AluOpType.add,
                )

                PRE = 2
                for btc in range(min(PRE, btc_used)):
                    emit_uchunk(btc)

                for t in range(frames):
                    if t % 4 == 0 and t // 4 + 2 < btc_used:
                        emit_uchunk(t // 4 + 2)
                    cur, nxt = t % 2, (t + 1) % 2
                    c_q = cp.tile([128, 512], F32, tag="c")
                    nc.sync.dma_start(c_q[:], C_d[t * 128:(t + 1) * 128, :])

                    if t == 0:
                        t1f = c_q[:]
                    else:
                        # col-tiled quads: the 4 output chunks run concurrently
                        # in the 4 column-groups of the PE array (M=32 each).
                        # kt ordered by (kt%4) so the stationary consumes the
                        # previous frame's spike blocks in production order.
                        r_ps = rps.tile([128, 512], F32, tag="r")
                        for pi, A_h in enumerate((A_hi, A_lo)):
                            for q in range(4):
                                for kk in range(4):
                                    kt = 4 * kk + q
                                    so = q * 128 + kk * 32
                                    first = pi == 0 and q == 0 and kk == 0
                                    last = pi == 1 and q == 3 and kk == 3
                                    for hc in range(HC):
                                        nc.tensor.matmul(
                                            r_ps[hc * 32:(hc + 1) * 32, :],
                                            spk_t[cur][:, so:so + 32],
                                            A_h[:, kt * N_HID + hc * 512: kt * N_HID + hc * 512 + 512],
                                            start=first,
                                            stop=last,
                                            tile_position=(0, hc * 32),
                                            skip_group_check=True,
                                        )
                        t1t = fs.tile([128, 512], F32, tag="t1", bufs=1)
                        t1f = t1t[:]

                    y_hm = fs.tile([128, 512], F32, tag="yhm", bufs=1)
                    # per-block pipeline: add-C, transpose, tanh (from PSUM),
                    # threshold -- each 128-wide block flows independently so
                    # the next frame's matmuls can start on early blocks.
                    for j in range(4):
                        blk = slice(j * 128, (j + 1) * 128)
                        if t > 0:
                            nc.vector.tensor_add(t1f[:, blk], r_ps[:, blk], c_q[:, blk])
                        tp = tps.tile([128, 128], F32, tag="tp", name="tp")
                        nc.tensor.transpose(tp[:], t1f[:, blk], eye_sb[:])
                        nc.scalar.activation(
                            y_hm[:, blk], tp[:], mybir.ActivationFunctionType.Tanh
                        )
                        nc.vector.tensor_tensor(
                            spk_t[nxt][:, blk], y_hm[:, blk], tmh_sb[:, blk],
                            op=mybir.AluOpType.is_gt,
                        )

                    # off the spike chain: mem' = y + h, then next h and 0.5-h
                    nc.vector.tensor_add(mem_t[nxt][:], y_hm[:], h_sb[:])
                    if t + 1 < frames:
                        nc.vector.tensor_scalar(
                            sph_sb[:], mem_t[nxt][:], THR, 0.5,
                            mybir.AluOpType.is_gt, mybir.AluOpType.mult,
                        )
                        nc.vector.tensor_scalar(
                            h1_sb[:], mem_t[nxt][:], 0.5, -0.5,
                            mybir.AluOpType.mult, mybir.AluOpType.add,
                        )
                        nc.vector.tensor_add(h_sb[:], h1_sb[:], sph_sb[:])
                        nc.vector.tensor_scalar(
                            tmh_sb[:], h_sb[:], -1.0, 0.5,
                            mybir.AluOpType.mult, mybir.AluOpType.add,
                        )

                    nc.sync.dma_start(memsT[t], mem_t[nxt][:])
                    nc.sync.dma_start(spikesT[t], spk_t[nxt][:])

    nc.compile()
    return nc


def _rn11(a):
    """Round fp32 to 11 mantissa bits (RNE) — matches TRN2 f32r operand rounding."""
    ai = np.asarray(a, np.float32).view(np.uint32).astype(np.uint64)
    keep = ai >> 12
    rem = ai & 0xFFF
    up = (rem > 0x800) | ((rem == 0x800) & ((keep & 1) == 1))
    return (((keep + up) << 12) & 0xFFFFFFFF).astype(np.uint32).view(np.float32)


def _host_prep(x, W_in, A, bias, mem_init, frames):
    """Build per-core input maps (shared arrays computed once)."""
    x = np.ascontiguousarray(x, dtype=np.float32)
    W_in = np.asarray(W_in, dtype=np.float32)
    A = np.asarray(A, dtype=np.float32)
    bias = np.asarray(bias, dtype=np.float32)
    mem_init = np.asarray(mem_init, dtype=np.float32)

    import ml_dtypes

    Apf = (ALPHA * A).reshape(KT, 128, N_HID).transpose(1, 0, 2).reshape(128, KT * N_HID)
    Apf = np.ascontiguousarray(Apf)
    Aph = Apf.astype(ml_dtypes.bfloat16)
    Apl = (Apf - Aph.astype(np.float32)).astype(ml_dtypes.bfloat16)

    W_aug = np.zeros((KIN, N_HID), dtype=np.float32)
    W_aug[:N_IN] = (1.0 - ALPHA) * W_in
    W_aug[N_IN] = (1.0 - ALPHA) * bias
    Wh = W_aug.astype(np.float16)
    Wl = (W_aug - Wh.astype(np.float32)).astype(np.float16)  # mostly subnormal: exact on PE
    Wph = np.ascontiguousarray(
        Wh.reshape(KC, 128, N_HID).transpose(1, 0, 2).reshape(128, KC * N_HID))
    Wpl = np.ascontiguousarray(
        Wl.reshape(KC, 128, N_HID).transpose(1, 0, 2).reshape(128, KC * N_HID))

    eye = np.eye(128, dtype=np.float32)

    in_maps = []
    for i in range(NCORES):
        xs = x[i * B:(i + 1) * B, :frames]            # [B, frames, N_IN]
        xTc = np.zeros((KIN, B * frames), dtype=np.float32)
        # xT[n, t*B + b] = x[b, t, n]
        xTc[:N_IN] = xs.transpose(2, 1, 0).reshape(N_IN, frames * B)
        xTc[N_IN] = 1.0
        if frames < FRAMES:
            full = np.zeros((KIN, BT), dtype=np.float32)
            full[:, : B * frames] = xTc
            xTc = full
        xTc16 = xTc.astype(np.float16)
        xTl16 = (xTc - xTc16.astype(np.float32)).astype(np.float16)
        ms = mem_init[i * B:(i + 1) * B]              # [B, N_HID]
        # hm packing: hm[p, q*128 + hc*32 + b] = mem[b, hc*512 + q*128 + p]
        m0 = ms.reshape(B, 4, 4, 128).transpose(3, 2, 1, 0).reshape(128, 512)
        in_maps.append(
            {
                "Aph": Aph,
                "Apl": Apl,
                "Wph": Wph,
                "Wpl": Wpl,
                "xTh": np.ascontiguousarray(xTc16),
                "xTl": np.ascontiguousarray(xTl16),
                "mem0": np.ascontiguousarray(m0),
                "eye": eye,
            }
        )
    return in_maps


def kernel(x, W_in, A, bias, mem_init):
    global LAST_RESULT
    frames = int(os.environ.get("ANNRC_FRAMES", FRAMES))

    if frames not in _NC_CACHE:
        _NC_CACHE[frames] = _build_nc(frames)
    nc = _NC_CACHE[frames]

    in_maps = _host_prep(x, W_in, A, bias, mem_init, frames)
    res = run_bass_kernel_spmd(nc, in_maps, core_ids=list(range(NCORES)))
    LAST_RESULT = res

    mems = np.empty((BATCH, frames, N_HID), dtype=np.float32)
    spikes = np.empty((BATCH, frames, N_HID), dtype=np.float32)
    for i in range(NCORES):
        out = res.results[i]
        mt = out["memsT"].reshape(frames, 128, 4, 4, B).transpose(4, 0, 3, 2, 1)
        mems[i * B:(i + 1) * B] = mt.reshape(B, frames, N_HID)
        sp = np.asarray(out["spikesT"], np.float32).reshape(frames, 128, 4, 4, B)
        spikes[i * B:(i + 1) * B] = sp.transpose(4, 0, 3, 2, 1).reshape(B, frames, N_HID)
    return mems, spikes

